# revision 51
# baseline (speedup 1.0000x reference)
"""Multi-head attention (RoPE + pos_bias + mask) Trainium2 Bass kernel.

Sharding: tensor-parallel over heads (2 heads per core, 8 cores), both
batch elements on every core.  Each core computes its heads' attention
and a partial o_proj (its slice of the contraction dim); the host sums
the 8 partials and adds b_o.

Design notes:
- fp16 operands everywhere off the PSUM accumulators (x, W, q/k/v, ex).
  Matmuls run 1 cycle/row; DVE elementwise gets the 2x 16-bit mode.
- DMA layouts are host-prepared so every big transfer is contiguous
  per partition (8KB descriptors): xT chunk-major, EB pre-transposed.
  First DMAs emitted are x chunk 0 + W_qk so the PE starts early.
  cos/sin tables are stored once per batch ([128,S], indexed mod S).
- Phase A rope: rotate_half is fused into the sin multiply (inputs
  read at a common base partition - a HW requirement - with the output
  written 32 partitions rotated; the sin table is host-permuted to the
  SOURCE rows).  Chunk 0's q/k rope chains run on DVE/GpSimd in
  parallel since they gate the first logits.
- The additive pos_bias+mask is replaced by a host-precomputed
  MULTIPLICATIVE table EB = exp(pos_bias*scale) (0 at masked entries).
  ACT exponentiates raw logits straight out of PSUM (one [128,1024]
  instruction covers both heads), then one fp16 multiply applies EB.
  exp carries a constant -4 bias so fp16 can't overflow; the shift
  cancels in the softmax ratio.
- V carries a 64-wide ones block, so the PV matmul replicates the
  softmax denominator into output partitions 64-127 for free (matmul
  cost is set by the moving dim).  1/den = exp(-ln(den)) on ACT (Exp
  and Ln share one table set; ACT cost is free-size based).
- Software-pipelined emission: every engine queue is in-order, so each
  block's tail work (normalize, o_proj, output DMA) is deferred into
  the NEXT block's kt loop via a filler queue, phase A's batch-1
  chunks are woven into the first attention block, and EB tiles for
  the next query chunk prefetch during the current one.  This keeps
  the PE dense enough that the HAM clock gate mostly stays at 8/8.
- PSUM budget (8 banks): logits/o_proj ring 2x[128,1024] (4), PV
  accumulators 2x[128,512] (2), qkv-projection ring 2x[128,512] (2).
- GPSIMD (otherwise idle) takes 3 of 16 EB multiplies per block.
"""
from collections import deque

import numpy as np
import ml_dtypes

import concourse.bass as bass
import concourse.mybir as mybir
import concourse.tile as tile
from concourse.bass_utils import run_bass_kernel_spmd

B, S, D, H, HD = 2, 2048, 1024, 16, 64
NCORES = 8
T = B * S            # 4096 tokens
KO = D // 128        # 8 contraction subtiles
QC = S // 512        # 4 query chunks per batch
KT = S // 128        # 16 key tiles per batch
EXP_SHIFT = -4.0     # uniform exp shift; cancels in softmax
SKEW = 3             # kt iterations the PV matmuls trail the exp/mul chain
GP_KTS = (1, 5, 9)  # kt slots whose EB multiply runs on GPSIMD

F32 = mybir.dt.float32
F16 = mybir.dt.float16
AF = mybir.ActivationFunctionType

TRACE = False
LAST_RESULT = None   # BassKernelResults of the most recent run (for profiling)

_waitfix_ctr = [0]


def _split_waits(nc, max_waits=1):
    """walrus in this environment accepts only one sync-wait command per
    instruction; TileContext emits several on some (notably the tail
    drain).  Move extras onto single-wait NoOps inserted just before, on
    the same engine queue — identical ordering semantics."""
    total = 0
    for fn in nc.m.functions:
        for bb in fn.blocks:
            out = []
            changed = False
            for ins in bb.instructions:
                si = ins.sync_info
                if si is not None and si.on_wait and len(si.on_wait) > max_waits:
                    waits = list(si.on_wait)
                    for w in waits[:-max_waits]:
                        _waitfix_ctr[0] += 1
                        n = mybir.InstNoOp(
                            name=f"I-waitfix-{_waitfix_ctr[0]}",
                            ins=[], outs=[], engine=ins.engine,
                        )
                        n.sync_info = mybir.SyncInfo(on_wait=[w], on_update=[])
                        out.append(n)
                        total += 1
                    ins.sync_info = mybir.SyncInfo(
                        on_wait=waits[-max_waits:],
                        on_update=list(si.on_update or []),
                    )
                    changed = True
                out.append(ins)
            if changed:
                bb.instructions = out
    return total


def _build(has_qkv_bias):
    nc = bass.Bass()
    xT = nc.declare_dram_parameter("xT", [128, T // 512, KO, 512], F16,
                                   isOutput=False)
    wqk = nc.declare_dram_parameter("wqk", [128, KO, 256], F16, isOutput=False)
    wv = nc.declare_dram_parameter("wv", [128, KO, 128], F16, isOutput=False)
    wo = nc.declare_dram_parameter("wo", [128, D], F16, isOutput=False)
    cos2 = nc.declare_dram_parameter("cos2", [128, S], F16, isOutput=False)
    sinsh = nc.declare_dram_parameter("sinsh", [128, S], F16,
                                  isOutput=False)
    ebd = nc.declare_dram_parameter("ebd", [QC, 4, 128, 4, 1024], F16,
                                    isOutput=False)
    if has_qkv_bias:
        wqkb = nc.declare_dram_parameter("wqkb", [2, 256], F16, isOutput=False)
        wvb = nc.declare_dram_parameter("wvb", [2, 128], F16, isOutput=False)
    outp = nc.declare_dram_parameter("out", [T, D], F32, isOutput=True)

    with tile.TileContext(nc) as tc:
        with (
            tc.tile_pool(name="const", bufs=1) as cst,
            tc.tile_pool(name="persist", bufs=1) as pers,
        ):
            wqk_sb = cst.tile([128, KO, 256], F16)
            wv_sb = cst.tile([128, KO, 128], F16)
            cos_sb = cst.tile([128, S], F16)
            sin_sb = cst.tile([128, S], F16)
            wo_sb = cst.tile([128, D], F16)
            nbias = cst.tile([128, 1], F32)
            nc.vector.memset(nbias[:], EXP_SHIFT)
            if has_qkv_bias:
                wqkb_sb = cst.tile([2, 256], F16)
                nc.sync.dma_start(wqkb_sb[:], wqkb[:])
                wvb_sb = cst.tile([2, 128], F16)
                nc.sync.dma_start(wvb_sb[:], wvb[:])
                ones2 = cst.tile([2, 512], F16)
                nc.vector.memset(ones2[:], 0.0)
                nc.vector.memset(ones2[0:1, :], 1.0)

            qT = pers.tile([128, T], F16)
            kT = pers.tile([128, T], F16)
            # v1[tok_part, group, head, 128]: cols 64-127 are ones, so the
            # PV matmul replicates the softmax denominator into output
            # partitions 64-127 for free
            v1 = pers.tile([128, 32, 2, 128], F16)
            nc.vector.memset(v1[:, :, :, 64:128], 1.0)
            valsT = pers.tile([128, T], F16)

            # warm the exp table load early so it overlaps phase A
            wtile = cst.tile([128, 2], F32)
            nc.vector.memset(wtile[:], 0.0)
            wout = cst.tile([128, 2], F16)
            nc.scalar.activation(wout[:], wtile[:], AF.Exp, bias=nbias[:])

            with (
                tc.tile_pool(name="pa", bufs=4) as pa,
                tc.tile_pool(name="xcp", bufs=2) as xcp,
                tc.tile_pool(name="ebp", bufs=2) as ebp,
                tc.tile_pool(name="ex0p", bufs=6) as ex0p,
                tc.tile_pool(name="exmp", bufs=SKEW + 4) as exmp,
                tc.tile_pool(name="pbn", bufs=4) as pbn,
                tc.tile_pool(name="obp", bufs=3) as obp,
                tc.tile_pool(name="plp", bufs=2, space="PSUM") as plp,
                tc.tile_pool(name="pvp", bufs=2, space="PSUM") as pvp,
                tc.tile_pool(name="pqp", bufs=2, space="PSUM") as pqp,
            ):
                fillers = deque()
                obx = [None]    # current o_proj staging tile (shared by n2)

                # ---------- phase A --------------------------------------
                def load_xc(ch):
                    xc = xcp.tile([128, KO, 512], F16, tag="xc")
                    nc.sync.dma_start(xc[:], xT[:, ch])
                    return xc

                def emit_chunk(ch, xc):
                    cs = ch * 512
                    cm = cs % S     # cos/sin tables repeat per batch
                    for m in range(2):          # 0 = q, 1 = k
                        pq = pqp.tile([128, 512], F32, tag="pq")
                        for ko in range(KO):
                            nc.tensor.matmul(
                                pq[:],
                                wqk_sb[:, ko, m * 128:(m + 1) * 128],
                                xc[:, ko], start=(ko == 0),
                                stop=(ko == KO - 1 and not has_qkv_bias))
                        if has_qkv_bias:
                            nc.tensor.matmul(
                                pq[:], wqkb_sb[:, m * 128:(m + 1) * 128],
                                ones2[:], start=False, stop=True)
                        sq = pa.tile([128, 512], F16, tag="sq")
                        nc.scalar.copy(sq[:], pq[:])
                        # rotate_half fused into the sin multiply: inputs
                        # read at a common base partition (HW requires it),
                        # output written 32 partitions rotated; sin table is
                        # host-permuted to match the SOURCE rows
                        rt = pa.tile([128, 512], F16, tag="rt")
                        for hb in range(2):
                            b0 = 64 * hb
                            nc.vector.tensor_mul(
                                out=rt[b0:b0 + 32, :],
                                in0=sq[b0 + 32:b0 + 64, :],
                                in1=sin_sb[b0 + 32:b0 + 64, cm:cm + 512])
                            nc.vector.tensor_mul(
                                out=rt[b0 + 32:b0 + 64, :],
                                in0=sq[b0:b0 + 32, :],
                                in1=sin_sb[b0:b0 + 32, cm:cm + 512])
                        t1 = pa.tile([128, 512], F16, tag="t1")
                        nc.vector.tensor_mul(
                            out=t1[:], in0=sq[:], in1=cos_sb[:, cm:cm + 512])
                        dst = qT if m == 0 else kT
                        nc.vector.tensor_add(
                            out=dst[:, cs:cs + 512], in0=t1[:], in1=rt[:])
                    for tt in range(4):         # v in [token, dim] layout
                        pv = pqp.tile([128, 512], F32, tag="pq")
                        for ko in range(KO):
                            nc.tensor.matmul(
                                pv[:, 0:128],
                                xc[:, ko, tt * 128:(tt + 1) * 128],
                                wv_sb[:, ko], start=(ko == 0),
                                stop=(ko == KO - 1 and not has_qkv_bias))
                        if has_qkv_bias:
                            nc.tensor.matmul(
                                pv[:, 0:128], ones2[:, 0:128],
                                wvb_sb[:], start=False, stop=True)
                        g = ch * 4 + tt
                        nc.vector.tensor_copy(
                            out=v1[:, g, :, 0:64],
                            in_=pv[:, 0:128].rearrange(
                                "p (h d) -> p h d", h=2))

                # ---------- EB tile prefetch --------------------------------
                def eb_alloc(qc):
                    ebt = ebp.tile([128, KT, 1024], F16, tag="eb",
                                   name=f"eb_{qc}")

                    def mk(k4):
                        def f():
                            nc.sync.dma_start(
                                ebt[:, 4 * k4:4 * k4 + 4], ebd[qc, k4])
                        return f

                    return ebt, [mk(k4) for k4 in range(4)]

                # ---------- block tail (deferred into next block) ----------
                def tail_steps(qc, b, pvt, lnds):
                    qtok = b * S + qc * 512
                    norms = []
                    lates = []
                    for hl in range(2):
                        def fnorm(hl=hl, lnd=lnds[hl]):
                            h0 = 64 * hl
                            rec = pbn.tile([64, 512], F32, tag="rec")
                            nc.scalar.activation(rec[:], lnd[:],
                                                 AF.Exp, scale=-1.0)
                            nc.vector.tensor_mul(
                                out=valsT[h0:h0 + 64, qtok:qtok + 512],
                                in0=pvt[hl][0:64, :], in1=rec[:])
                        norms.append(fnorm)
                    for tt in range(4):
                        for n2 in range(2):
                            def foproj(tt=tt, n2=n2):
                                r0 = qtok + tt * 128
                                po = pqp.tile([128, 512], F32, tag="pq",
                                              name=f"po_{qc}_{b}_{tt}_{n2}")
                                nc.tensor.matmul(
                                    po[:],
                                    valsT[:, r0:r0 + 128],
                                    wo_sb[:, n2 * 512:(n2 + 1) * 512],
                                    start=True, stop=True)
                                if n2 == 0:
                                    obx[0] = obp.tile(
                                        [128, 1024], F32, tag="ob",
                                        name=f"ob_{qc}_{b}_{tt}")
                                ob = obx[0]
                                nc.vector.tensor_copy(
                                    out=ob[:, n2 * 512:(n2 + 1) * 512],
                                    in_=po[:])
                                if n2 == 1:
                                    nc.sync.dma_start(
                                        outp[r0:r0 + 128, :], ob[:])
                            lates.append(foproj)
                    return norms, lates

                # ---------- one attention block ----------------------------
                def block(qc, b, ebt):
                    qs = qc * 512
                    qtok = b * S + qs
                    pvt = [pvp.tile([128, 512], F32, tag="pv",
                                    name=f"pv_{qc}_{b}_{hl}")
                           for hl in range(2)]
                    pend = []

                    def flush_one():
                        fkt, fex = pend.pop(0)
                        for hl in range(2):
                            nc.tensor.matmul(
                                pvt[hl][:],
                                v1[:, b * 16 + fkt, hl],
                                fex[:, hl * 512:(hl + 1) * 512],
                                start=(fkt == 0), stop=(fkt == KT - 1),
                                skip_group_check=True)

                    for kt in range(KT):
                        ktok = b * S + kt * 128
                        pl = plp.tile([128, 1024], F32, tag="pl")
                        for hl in range(2):
                            h0 = 64 * hl
                            # the two heads run CONCURRENTLY in disjoint PE
                            # row groups (row-tiling), halving logits time
                            nc.tensor.matmul(
                                pl[:, hl * 512:(hl + 1) * 512],
                                kT[h0:h0 + 64, ktok:ktok + 128],
                                qT[h0:h0 + 64, qtok:qtok + 512],
                                start=True, stop=True,
                                tile_position=(64 * hl, 0),
                                skip_group_check=True)
                        ex0 = ex0p.tile([128, 1024], F16, tag="ex0")
                        nc.scalar.activation(ex0[:], pl[:], AF.Exp,
                                             bias=nbias[:])
                        exm = exmp.tile([128, 1024], F16, tag="exm")
                        eng = nc.gpsimd if kt in GP_KTS else nc.vector
                        eng.tensor_mul(out=exm[:], in0=ex0[:], in1=ebt[:, kt])
                        pend.append((kt, exm))
                        while len(pend) > SKEW:
                            flush_one()
                        if fillers:
                            fillers.popleft()()
                    while pend:
                        flush_one()
                    # ln(den) for both heads now (ACT); exp(-ln) + normalize
                    # deferred into the next block via the filler queue
                    lnds = []
                    for hl in range(2):
                        lnd = pbn.tile([64, 512], F32, tag="lnd")
                        nc.scalar.activation(lnd[:], pvt[hl][64:128, :],
                                             AF.Ln)
                        lnds.append(lnd)
                    norms, lates = tail_steps(qc, b, pvt, lnds)
                    for fstep in reversed(norms):
                        fillers.appendleft(fstep)
                    fillers.extend(lates)

                # ---------- emission timeline ------------------------------
                xc_cur = load_xc(0)
                nc.sync.dma_start(wqk_sb[:], wqk[:])
                nc.sync.dma_start(wv_sb[:], wv[:])
                nc.sync.dma_start(cos_sb[:], cos2[:])
                nc.sync.dma_start(sin_sb[:], sinsh[:])
                eb0, eb0_steps = eb_alloc(0)
                eb0_steps[0]()
                wo_dma = [False]
                for ch in range(T // 512):     # phase A: serial, PE-dense
                    xc_nxt = load_xc(ch + 1) if ch + 1 < T // 512 else None
                    emit_chunk(ch, xc_cur)
                    xc_cur = xc_nxt
                    if ch + 1 < len(eb0_steps):  # EB(qc0) prefetch during A
                        eb0_steps[ch + 1]()
                    elif not wo_dma[0]:
                        nc.sync.dma_start(wo_sb[:], wo[:])
                        wo_dma[0] = True

                eb_cur = eb0
                for qc in range(QC):
                    for b in range(2):
                        if b == 0 and qc + 1 < QC:
                            nxt, steps = eb_alloc(qc + 1)
                            fillers.extend(steps)
                        block(qc, b, eb_cur)
                        if b == 1 and qc + 1 < QC:
                            eb_cur = nxt
                while fillers:                 # last block's tail + leftovers
                    fillers.popleft()()
                # clock-holding tail ballast: GpSimd's in-order queue reaches
                # these after its last real op (mid final block), covering
                # the tail window so the HAM gate stays at full clock while
                # the serial tail chain drains.  cos_sb is dead after phase
                # A, so this costs no SBUF.
                for _ in range(12):
                    nc.gpsimd.memset(cos_sb[:, 0:1024], 0.0)

    _split_waits(nc)
    return nc


_nc_cache = {}


def _get_nc(has_qkv_bias):
    if has_qkv_bias not in _nc_cache:
        _nc_cache[has_qkv_bias] = _build(has_qkv_bias)
    return _nc_cache[has_qkv_bias]


def _prep_inputs(x, pos_bias, sinusoidal_pos, mask, W_qkv, b_qkv, W_o, b_o):
    """Build the 8 per-core input maps (all host-side layout prep)."""
    x = np.asarray(x, np.float32)
    pos_bias = np.asarray(pos_bias, np.float32)
    sp = np.asarray(sinusoidal_pos, np.float32)[0, 0]        # [S, HD]
    mask = np.asarray(mask)
    W_qkv = np.asarray(W_qkv, np.float32)
    b_qkv = np.asarray(b_qkv, np.float32)
    W_o = np.asarray(W_o, np.float32)

    f16 = ml_dtypes.float16 if hasattr(ml_dtypes, "float16") else np.float16
    scale = np.float32(1.0 / np.sqrt(HD))
    has_bias = bool(np.any(b_qkv))

    xflat = x.reshape(T, D)
    xT_np = np.ascontiguousarray(
        xflat.T.reshape(KO, 128, T // 512, 512).transpose(1, 2, 0, 3)
    ).astype(f16)

    cos_t = np.cos(sp).T.astype(np.float32)                   # [HD, S]
    sin_t = np.sin(sp).T.astype(np.float32)
    cos2_np = np.ascontiguousarray(np.tile(cos_t, (2, 1))).astype(f16)
    # sign folded for the DESTINATION row, then 32-row halves swapped so
    # the kernel's fused rotate-mul reads sin at the SOURCE row base
    sinsh64 = np.concatenate([-sin_t[:HD // 2], sin_t[HD // 2:]], axis=0)
    sinshP = np.concatenate([sinsh64[HD // 2:], sinsh64[:HD // 2]], axis=0)
    sinsh_np = np.ascontiguousarray(np.tile(sinshP, (2, 1))).astype(f16)

    maskT0 = (mask[0, 0].T == 0)                              # [S(k), S(q)]

    # per-head W rows: feature f = h*192 + j (j<64 q, <128 k, <192 v)
    Wh = W_qkv.reshape(H, 3 * HD, D)
    bh = b_qkv.reshape(H, 3 * HD)

    in_maps = []
    for c in range(NCORES):
        h0, h1 = 2 * c, 2 * c + 1
        # q rows scaled by 1/sqrt(HD); k rows unscaled
        Wqk_c = np.concatenate([
            Wh[h0, 0:HD] * scale, Wh[h1, 0:HD] * scale,
            Wh[h0, HD:2 * HD], Wh[h1, HD:2 * HD]], axis=0)    # [256, D]
        Wv_c = np.concatenate([Wh[h0, 2 * HD:], Wh[h1, 2 * HD:]], axis=0)

        wqk_np = np.ascontiguousarray(
            Wqk_c.T.reshape(KO, 128, 256).transpose(1, 0, 2)).astype(f16)
        wv_np = np.ascontiguousarray(
            Wv_c.T.reshape(KO, 128, 128).transpose(1, 0, 2)).astype(f16)
        wo_np = np.ascontiguousarray(
            W_o[:, 128 * c:128 * (c + 1)].T).astype(f16)       # [128, D]

        # multiplicative bias table: exp(pos_bias*scale), 0 where masked,
        # in logits-T layout [k, q]; split [qc, kt, 128, (hl 512 | 512)]
        ebd_np = np.empty((QC, KT, 128, 1024), f16)
        for hl in range(2):
            eb = np.exp(pos_bias[0, 2 * c + hl].T * scale)
            eb[maskT0] = 0.0
            ebd_np[:, :, :, hl * 512:(hl + 1) * 512] = (
                eb.reshape(KT, 128, QC, 512).transpose(2, 0, 1, 3)
            ).astype(f16)
        ebd_np = np.ascontiguousarray(
            ebd_np.reshape(QC, 4, 4, 128, 1024).transpose(0, 1, 3, 2, 4))

        m = {
            "xT": xT_np, "wqk": wqk_np, "wv": wv_np, "wo": wo_np,
            "cos2": cos2_np, "sinsh": sinsh_np, "ebd": ebd_np,
        }
        if has_bias:
            bqk_c = np.concatenate([
                bh[h0, 0:HD] * scale, bh[h1, 0:HD] * scale,
                bh[h0, HD:2 * HD], bh[h1, HD:2 * HD]], axis=0)
            bv_c = np.concatenate([bh[h0, 2 * HD:], bh[h1, 2 * HD:]], axis=0)
            wqkb_np = np.zeros((2, 256), np.float32)
            wqkb_np[0] = bqk_c
            wvb_np = np.zeros((2, 128), np.float32)
            wvb_np[0] = bv_c
            m["wqkb"] = wqkb_np.astype(f16)
            m["wvb"] = wvb_np.astype(f16)
        in_maps.append(m)
    return in_maps, has_bias


def _ensure_profile_hook():
    """Register the axon NTFF profiling hook if the image lacks
    antenv.axon_hooks (needed only for TRACE=True runs)."""
    import sys
    import types
    try:
        from antenv.axon_hooks import get_axon_ntff_profile_hook  # noqa
        return
    except ImportError:
        pass
    try:
        from trn_agent_boot.trn_boot import _ntff_profile_via_ctypes
        hook = _ntff_profile_via_ctypes("/opt/axon/libaxon_pjrt.so")
        mod = types.ModuleType("antenv.axon_hooks")
        mod.get_axon_ntff_profile_hook = lambda: hook
        mod.set_axon_ntff_profile_hook = lambda h: None
        sys.modules["antenv.axon_hooks"] = mod
    except Exception:
        pass


def kernel(x, pos_bias, sinusoidal_pos, mask, W_qkv, b_qkv, W_o, b_o):
    global LAST_RESULT
    if TRACE:
        _ensure_profile_hook()
    in_maps, has_bias = _prep_inputs(x, pos_bias, sinusoidal_pos, mask,
                                     W_qkv, b_qkv, W_o, b_o)
    nc = _get_nc(has_bias)
    try:
        r = run_bass_kernel_spmd(nc, in_maps, list(range(NCORES)),
                                 trace=TRACE)
    except Exception:
        # occasional transient NRT device errors — retry once
        r = run_bass_kernel_spmd(nc, in_maps, list(range(NCORES)),
                                 trace=TRACE)
    LAST_RESULT = r
    acc = np.zeros((T, D), np.float64)
    for c in range(NCORES):
        acc += r.results[c]["out"].astype(np.float64)
    out = (acc + np.asarray(b_o, np.float32).astype(np.float64)).astype(
        np.float32)
    return out.reshape(B, S, D)



# revision 53
# speedup vs baseline: 1.0920x; 1.0920x over previous
"""Multi-head attention (RoPE + pos_bias + mask) Trainium2 Bass kernel.

Sharding: tensor-parallel over heads (2 heads per core, 8 cores), both
batch elements on every core.  Each core computes its heads' attention
and a partial o_proj (its slice of the contraction dim); the host sums
the 8 partials and adds b_o.

Design notes:
- fp16 operands everywhere off the PSUM accumulators (x, W, q/k/v, ex).
  Matmuls run 1 cycle/row; DVE elementwise gets the 2x 16-bit mode.
- DMA layouts are host-prepared so every big transfer is contiguous
  per partition (8KB descriptors): xT chunk-major, EB pre-transposed.
  First DMAs emitted are x chunk 0 + W_qk so the PE starts early.
  cos/sin tables are stored once per batch ([128,S], indexed mod S).
- Phase A rope: rotate_half is fused into the sin multiply (inputs
  read at a common base partition - a HW requirement - with the output
  written 32 partitions rotated; the sin table is host-permuted to the
  SOURCE rows).  Chunk 0's q/k rope chains run on DVE/GpSimd in
  parallel since they gate the first logits.
- The additive pos_bias+mask is replaced by a host-precomputed
  MULTIPLICATIVE table EB = exp(pos_bias*scale) (0 at masked entries).
  ACT exponentiates raw logits straight out of PSUM (one [128,1024]
  instruction covers both heads), then one fp16 multiply applies EB.
  exp carries a constant -4 bias so fp16 can't overflow; the shift
  cancels in the softmax ratio.
- V carries a 64-wide ones block, so the PV matmul replicates the
  softmax denominator into output partitions 64-127 for free (matmul
  cost is set by the moving dim).  1/den = exp(-ln(den)) on ACT (Exp
  and Ln share one table set; ACT cost is free-size based).
- Software-pipelined emission: every engine queue is in-order, so each
  block's tail work (normalize, o_proj, output DMA) is deferred into
  the NEXT block's kt loop via a filler queue, phase A's batch-1
  chunks are woven into the first attention block, and EB tiles for
  the next query chunk prefetch during the current one.  This keeps
  the PE dense enough that the HAM clock gate mostly stays at 8/8.
- PSUM budget (8 banks): logits/o_proj ring 2x[128,1024] (4), PV
  accumulators 2x[128,512] (2), qkv-projection ring 2x[128,512] (2).
- GPSIMD (otherwise idle) takes 3 of 16 EB multiplies per block.
"""
from collections import deque

import numpy as np
import ml_dtypes

import concourse.bass as bass
import concourse.mybir as mybir
import concourse.tile as tile
from concourse.bass_utils import run_bass_kernel_spmd

B, S, D, H, HD = 2, 2048, 1024, 16, 64
NCORES = 8
T = B * S            # 4096 tokens
KO = D // 128        # 8 contraction subtiles
QC = S // 512        # 4 query chunks per batch
KT = S // 128        # 16 key tiles per batch
EXP_SHIFT = -4.0     # uniform exp shift; cancels in softmax
SKEW = 3             # kt iterations the PV matmuls trail the exp/mul chain
GP_KTS = (1, 5, 9)  # kt slots whose EB multiply runs on GPSIMD

F32 = mybir.dt.float32
F16 = mybir.dt.float16
AF = mybir.ActivationFunctionType

TRACE = False
LAST_RESULT = None   # BassKernelResults of the most recent run (for profiling)

_waitfix_ctr = [0]


def _split_waits(nc, max_waits=1):
    """walrus in this environment accepts only one sync-wait command per
    instruction; TileContext emits several on some (notably the tail
    drain).  Move extras onto single-wait NoOps inserted just before, on
    the same engine queue — identical ordering semantics."""
    total = 0
    for fn in nc.m.functions:
        for bb in fn.blocks:
            out = []
            changed = False
            for ins in bb.instructions:
                si = ins.sync_info
                if si is not None and si.on_wait and len(si.on_wait) > max_waits:
                    waits = list(si.on_wait)
                    for w in waits[:-max_waits]:
                        _waitfix_ctr[0] += 1
                        n = mybir.InstNoOp(
                            name=f"I-waitfix-{_waitfix_ctr[0]}",
                            ins=[], outs=[], engine=ins.engine,
                        )
                        n.sync_info = mybir.SyncInfo(on_wait=[w], on_update=[])
                        out.append(n)
                        total += 1
                    ins.sync_info = mybir.SyncInfo(
                        on_wait=waits[-max_waits:],
                        on_update=list(si.on_update or []),
                    )
                    changed = True
                out.append(ins)
            if changed:
                bb.instructions = out
    return total


def _build(has_qkv_bias):
    nc = bass.Bass()
    xT = nc.declare_dram_parameter("xT", [128, T // 512, KO, 512], F16,
                                   isOutput=False)
    wqk = nc.declare_dram_parameter("wqk", [128, KO, 256], F16, isOutput=False)
    wv = nc.declare_dram_parameter("wv", [128, KO, 128], F16, isOutput=False)
    wo = nc.declare_dram_parameter("wo", [128, D], F16, isOutput=False)
    cos2 = nc.declare_dram_parameter("cos2", [128, S], F16, isOutput=False)
    sinsh = nc.declare_dram_parameter("sinsh", [128, S], F16,
                                  isOutput=False)
    ebd = nc.declare_dram_parameter("ebd", [QC, 4, 128, 4, 1024], F16,
                                    isOutput=False)
    if has_qkv_bias:
        wqkb = nc.declare_dram_parameter("wqkb", [2, 256], F16, isOutput=False)
        wvb = nc.declare_dram_parameter("wvb", [2, 128], F16, isOutput=False)
    outp = nc.declare_dram_parameter("out", [T, D], F32, isOutput=True)

    with tile.TileContext(nc) as tc:
        with (
            tc.tile_pool(name="const", bufs=1) as cst,
            tc.tile_pool(name="persist", bufs=1) as pers,
        ):
            wqk_sb = cst.tile([128, KO, 256], F16)
            wv_sb = cst.tile([128, KO, 128], F16)
            cos_sb = cst.tile([128, S], F16)
            sin_sb = cst.tile([128, S], F16)
            wo_sb = cst.tile([128, D], F16)
            nbias = cst.tile([128, 1], F32)
            nc.vector.memset(nbias[:], EXP_SHIFT)
            if has_qkv_bias:
                wqkb_sb = cst.tile([2, 256], F16)
                nc.sync.dma_start(wqkb_sb[:], wqkb[:])
                wvb_sb = cst.tile([2, 128], F16)
                nc.sync.dma_start(wvb_sb[:], wvb[:])
                ones2 = cst.tile([2, 512], F16)
                nc.vector.memset(ones2[:], 0.0)
                nc.vector.memset(ones2[0:1, :], 1.0)

            qT = pers.tile([128, T], F16)
            kT = pers.tile([128, T], F16)
            # v1[tok_part, group, head, 128]: cols 64-127 are ones, so the
            # PV matmul replicates the softmax denominator into output
            # partitions 64-127 for free
            v1 = pers.tile([128, 32, 2, 128], F16)
            nc.vector.memset(v1[:, :, :, 64:128], 1.0)
            valsT = pers.tile([128, T], F16)

            # warm the exp table load early so it overlaps phase A
            wtile = cst.tile([128, 2], F32)
            nc.vector.memset(wtile[:], 0.0)
            wout = cst.tile([128, 2], F16)
            nc.scalar.activation(wout[:], wtile[:], AF.Exp, bias=nbias[:])

            with (
                tc.tile_pool(name="pa", bufs=4) as pa,
                tc.tile_pool(name="xcp", bufs=2) as xcp,
                tc.tile_pool(name="ebp", bufs=2) as ebp,
                tc.tile_pool(name="ex0p", bufs=6) as ex0p,
                tc.tile_pool(name="exmp", bufs=SKEW + 4) as exmp,
                tc.tile_pool(name="pbn", bufs=4) as pbn,
                tc.tile_pool(name="obp", bufs=4) as obp,
                tc.tile_pool(name="plp", bufs=2, space="PSUM") as plp,
                tc.tile_pool(name="pvp", bufs=2, space="PSUM") as pvp,
                tc.tile_pool(name="pqp", bufs=2, space="PSUM") as pqp,
            ):
                fillers = deque()
                obx = [None]    # current o_proj staging tile (shared by n2)

                # ---------- phase A --------------------------------------
                def load_xc(ch):
                    xc = xcp.tile([128, KO, 512], F16, tag="xc")
                    nc.sync.dma_start(xc[:], xT[:, ch])
                    return xc

                def emit_chunk(ch, xc):
                    cs = ch * 512
                    cm = cs % S     # cos/sin tables repeat per batch
                    for m in range(2):          # 0 = q, 1 = k
                        pq = pqp.tile([128, 512], F32, tag="pq")
                        for ko in range(KO):
                            nc.tensor.matmul(
                                pq[:],
                                wqk_sb[:, ko, m * 128:(m + 1) * 128],
                                xc[:, ko], start=(ko == 0),
                                stop=(ko == KO - 1 and not has_qkv_bias))
                        if has_qkv_bias:
                            nc.tensor.matmul(
                                pq[:], wqkb_sb[:, m * 128:(m + 1) * 128],
                                ones2[:], start=False, stop=True)
                        sq = pa.tile([128, 512], F16, tag="sq")
                        nc.scalar.copy(sq[:], pq[:])
                        # rotate_half fused into the sin multiply: inputs
                        # read at a common base partition (HW requires it),
                        # output written 32 partitions rotated; sin table is
                        # host-permuted to match the SOURCE rows
                        rt = pa.tile([128, 512], F16, tag="rt")
                        for hb in range(2):
                            b0 = 64 * hb
                            nc.vector.tensor_mul(
                                out=rt[b0:b0 + 32, :],
                                in0=sq[b0 + 32:b0 + 64, :],
                                in1=sin_sb[b0 + 32:b0 + 64, cm:cm + 512])
                            nc.vector.tensor_mul(
                                out=rt[b0 + 32:b0 + 64, :],
                                in0=sq[b0:b0 + 32, :],
                                in1=sin_sb[b0:b0 + 32, cm:cm + 512])
                        t1 = pa.tile([128, 512], F16, tag="t1")
                        nc.vector.tensor_mul(
                            out=t1[:], in0=sq[:], in1=cos_sb[:, cm:cm + 512])
                        dst = qT if m == 0 else kT
                        nc.vector.tensor_add(
                            out=dst[:, cs:cs + 512], in0=t1[:], in1=rt[:])
                    for tt in range(4):         # v in [token, dim] layout
                        pv = pqp.tile([128, 512], F32, tag="pq")
                        for ko in range(KO):
                            nc.tensor.matmul(
                                pv[:, 0:128],
                                xc[:, ko, tt * 128:(tt + 1) * 128],
                                wv_sb[:, ko], start=(ko == 0),
                                stop=(ko == KO - 1 and not has_qkv_bias))
                        if has_qkv_bias:
                            nc.tensor.matmul(
                                pv[:, 0:128], ones2[:, 0:128],
                                wvb_sb[:], start=False, stop=True)
                        g = ch * 4 + tt
                        nc.vector.tensor_copy(
                            out=v1[:, g, :, 0:64],
                            in_=pv[:, 0:128].rearrange(
                                "p (h d) -> p h d", h=2))

                # ---------- EB tile prefetch --------------------------------
                def eb_alloc(qc):
                    ebt = ebp.tile([128, KT, 1024], F16, tag="eb",
                                   name=f"eb_{qc}")

                    def mk(k4):
                        def f():
                            nc.sync.dma_start(
                                ebt[:, 4 * k4:4 * k4 + 4], ebd[qc, k4])
                        return f

                    return ebt, [mk(k4) for k4 in range(4)]

                # ---------- block tail (deferred into next block) ----------
                def tail_steps(qc, b, pvt, lnds):
                    qtok = b * S + qc * 512
                    norms = []
                    lates = []
                    for hl in range(2):
                        def fnorm(hl=hl, lnd=lnds[hl]):
                            h0 = 64 * hl
                            rec = pbn.tile([64, 512], F32, tag="rec")
                            nc.scalar.activation(rec[:], lnd[:],
                                                 AF.Exp, scale=-1.0)
                            nc.vector.tensor_mul(
                                out=valsT[h0:h0 + 64, qtok:qtok + 512],
                                in0=pvt[hl][0:64, :], in1=rec[:])
                        norms.append(fnorm)
                    for tt in range(4):
                        for n2 in range(2):
                            def foproj(tt=tt, n2=n2):
                                r0 = qtok + tt * 128
                                po = pqp.tile([128, 512], F32, tag="pq",
                                              name=f"po_{qc}_{b}_{tt}_{n2}")
                                nc.tensor.matmul(
                                    po[:],
                                    valsT[:, r0:r0 + 128],
                                    wo_sb[:, n2 * 512:(n2 + 1) * 512],
                                    start=True, stop=True)
                                if n2 == 0:
                                    obx[0] = obp.tile(
                                        [128, 1024], F32, tag="ob",
                                        name=f"ob_{qc}_{b}_{tt}")
                                ob = obx[0]
                                nc.vector.tensor_copy(
                                    out=ob[:, n2 * 512:(n2 + 1) * 512],
                                    in_=po[:])
                                if n2 == 1:
                                    nc.sync.dma_start(
                                        outp[r0:r0 + 128, :], ob[:])
                            lates.append(foproj)
                    return norms, lates

                # ---------- one attention block ----------------------------
                def block(qc, b, ebt):
                    qs = qc * 512
                    qtok = b * S + qs
                    pvt = [pvp.tile([128, 512], F32, tag="pv",
                                    name=f"pv_{qc}_{b}_{hl}")
                           for hl in range(2)]
                    pend = []

                    def flush_one():
                        fkt, fex = pend.pop(0)
                        for hl in range(2):
                            nc.tensor.matmul(
                                pvt[hl][:],
                                v1[:, b * 16 + fkt, hl],
                                fex[:, hl * 512:(hl + 1) * 512],
                                start=(fkt == 0), stop=(fkt == KT - 1),
                                skip_group_check=True)

                    for kt in range(KT):
                        ktok = b * S + kt * 128
                        pl = plp.tile([128, 1024], F32, tag="pl")
                        for hl in range(2):
                            h0 = 64 * hl
                            # the two heads run CONCURRENTLY in disjoint PE
                            # row groups (row-tiling), halving logits time
                            nc.tensor.matmul(
                                pl[:, hl * 512:(hl + 1) * 512],
                                kT[h0:h0 + 64, ktok:ktok + 128],
                                qT[h0:h0 + 64, qtok:qtok + 512],
                                start=True, stop=True,
                                tile_position=(64 * hl, 0),
                                skip_group_check=True)
                        ex0 = ex0p.tile([128, 1024], F16, tag="ex0")
                        nc.scalar.activation(ex0[:], pl[:], AF.Exp,
                                             bias=nbias[:])
                        exm = exmp.tile([128, 1024], F16, tag="exm")
                        eng = nc.gpsimd if kt in GP_KTS else nc.vector
                        eng.tensor_mul(out=exm[:], in0=ex0[:], in1=ebt[:, kt])
                        pend.append((kt, exm))
                        while len(pend) > SKEW:
                            flush_one()
                        if fillers:
                            fillers.popleft()()
                    while pend:
                        flush_one()
                    # ln(den) for both heads now (ACT); exp(-ln) + normalize
                    # deferred into the next block via the filler queue
                    lnds = []
                    for hl in range(2):
                        lnd = pbn.tile([64, 512], F32, tag="lnd")
                        nc.scalar.activation(lnd[:], pvt[hl][64:128, :],
                                             AF.Ln)
                        lnds.append(lnd)
                    norms, lates = tail_steps(qc, b, pvt, lnds)
                    for fstep in reversed(norms):
                        fillers.appendleft(fstep)
                    fillers.extend(lates)

                # ---------- emission timeline ------------------------------
                xc_cur = load_xc(0)
                nc.sync.dma_start(wqk_sb[:], wqk[:])
                nc.sync.dma_start(wv_sb[:], wv[:])
                nc.sync.dma_start(cos_sb[:], cos2[:])
                nc.sync.dma_start(sin_sb[:], sinsh[:])
                eb0, eb0_steps = eb_alloc(0)
                eb0_steps[0]()
                wo_dma = [False]
                for ch in range(T // 512):     # phase A: serial, PE-dense
                    xc_nxt = load_xc(ch + 1) if ch + 1 < T // 512 else None
                    emit_chunk(ch, xc_cur)
                    xc_cur = xc_nxt
                    if ch + 1 < len(eb0_steps):  # EB(qc0) prefetch during A
                        eb0_steps[ch + 1]()
                    elif not wo_dma[0]:
                        nc.sync.dma_start(wo_sb[:], wo[:])
                        wo_dma[0] = True

                eb_cur = eb0
                for qc in range(QC):
                    for b in range(2):
                        if b == 0 and qc + 1 < QC:
                            nxt, steps = eb_alloc(qc + 1)
                            fillers.extend(steps)
                        block(qc, b, eb_cur)
                        if b == 1 and qc + 1 < QC:
                            eb_cur = nxt
                while fillers:                 # last block's tail + leftovers
                    fillers.popleft()()

    _split_waits(nc)
    return nc


_nc_cache = {}


def _get_nc(has_qkv_bias):
    if has_qkv_bias not in _nc_cache:
        _nc_cache[has_qkv_bias] = _build(has_qkv_bias)
    return _nc_cache[has_qkv_bias]


def _prep_inputs(x, pos_bias, sinusoidal_pos, mask, W_qkv, b_qkv, W_o, b_o):
    """Build the 8 per-core input maps (all host-side layout prep)."""
    x = np.asarray(x, np.float32)
    pos_bias = np.asarray(pos_bias, np.float32)
    sp = np.asarray(sinusoidal_pos, np.float32)[0, 0]        # [S, HD]
    mask = np.asarray(mask)
    W_qkv = np.asarray(W_qkv, np.float32)
    b_qkv = np.asarray(b_qkv, np.float32)
    W_o = np.asarray(W_o, np.float32)

    f16 = ml_dtypes.float16 if hasattr(ml_dtypes, "float16") else np.float16
    scale = np.float32(1.0 / np.sqrt(HD))
    has_bias = bool(np.any(b_qkv))

    xflat = x.reshape(T, D)
    xT_np = np.ascontiguousarray(
        xflat.T.reshape(KO, 128, T // 512, 512).transpose(1, 2, 0, 3)
    ).astype(f16)

    cos_t = np.cos(sp).T.astype(np.float32)                   # [HD, S]
    sin_t = np.sin(sp).T.astype(np.float32)
    cos2_np = np.ascontiguousarray(np.tile(cos_t, (2, 1))).astype(f16)
    # sign folded for the DESTINATION row, then 32-row halves swapped so
    # the kernel's fused rotate-mul reads sin at the SOURCE row base
    sinsh64 = np.concatenate([-sin_t[:HD // 2], sin_t[HD // 2:]], axis=0)
    sinshP = np.concatenate([sinsh64[HD // 2:], sinsh64[:HD // 2]], axis=0)
    sinsh_np = np.ascontiguousarray(np.tile(sinshP, (2, 1))).astype(f16)

    maskT0 = (mask[0, 0].T == 0)                              # [S(k), S(q)]

    # per-head W rows: feature f = h*192 + j (j<64 q, <128 k, <192 v)
    Wh = W_qkv.reshape(H, 3 * HD, D)
    bh = b_qkv.reshape(H, 3 * HD)

    in_maps = []
    for c in range(NCORES):
        h0, h1 = 2 * c, 2 * c + 1
        # q rows scaled by 1/sqrt(HD); k rows unscaled
        Wqk_c = np.concatenate([
            Wh[h0, 0:HD] * scale, Wh[h1, 0:HD] * scale,
            Wh[h0, HD:2 * HD], Wh[h1, HD:2 * HD]], axis=0)    # [256, D]
        Wv_c = np.concatenate([Wh[h0, 2 * HD:], Wh[h1, 2 * HD:]], axis=0)

        wqk_np = np.ascontiguousarray(
            Wqk_c.T.reshape(KO, 128, 256).transpose(1, 0, 2)).astype(f16)
        wv_np = np.ascontiguousarray(
            Wv_c.T.reshape(KO, 128, 128).transpose(1, 0, 2)).astype(f16)
        wo_np = np.ascontiguousarray(
            W_o[:, 128 * c:128 * (c + 1)].T).astype(f16)       # [128, D]

        # multiplicative bias table: exp(pos_bias*scale), 0 where masked,
        # in logits-T layout [k, q]; split [qc, kt, 128, (hl 512 | 512)]
        ebd_np = np.empty((QC, KT, 128, 1024), f16)
        for hl in range(2):
            eb = np.exp(pos_bias[0, 2 * c + hl].T * scale)
            eb[maskT0] = 0.0
            ebd_np[:, :, :, hl * 512:(hl + 1) * 512] = (
                eb.reshape(KT, 128, QC, 512).transpose(2, 0, 1, 3)
            ).astype(f16)
        ebd_np = np.ascontiguousarray(
            ebd_np.reshape(QC, 4, 4, 128, 1024).transpose(0, 1, 3, 2, 4))

        m = {
            "xT": xT_np, "wqk": wqk_np, "wv": wv_np, "wo": wo_np,
            "cos2": cos2_np, "sinsh": sinsh_np, "ebd": ebd_np,
        }
        if has_bias:
            bqk_c = np.concatenate([
                bh[h0, 0:HD] * scale, bh[h1, 0:HD] * scale,
                bh[h0, HD:2 * HD], bh[h1, HD:2 * HD]], axis=0)
            bv_c = np.concatenate([bh[h0, 2 * HD:], bh[h1, 2 * HD:]], axis=0)
            wqkb_np = np.zeros((2, 256), np.float32)
            wqkb_np[0] = bqk_c
            wvb_np = np.zeros((2, 128), np.float32)
            wvb_np[0] = bv_c
            m["wqkb"] = wqkb_np.astype(f16)
            m["wvb"] = wvb_np.astype(f16)
        in_maps.append(m)
    return in_maps, has_bias


def _ensure_profile_hook():
    """Register the axon NTFF profiling hook if the image lacks
    antenv.axon_hooks (needed only for TRACE=True runs)."""
    import sys
    import types
    try:
        from antenv.axon_hooks import get_axon_ntff_profile_hook  # noqa
        return
    except ImportError:
        pass
    try:
        from trn_agent_boot.trn_boot import _ntff_profile_via_ctypes
        hook = _ntff_profile_via_ctypes("/opt/axon/libaxon_pjrt.so")
        mod = types.ModuleType("antenv.axon_hooks")
        mod.get_axon_ntff_profile_hook = lambda: hook
        mod.set_axon_ntff_profile_hook = lambda h: None
        sys.modules["antenv.axon_hooks"] = mod
    except Exception:
        pass


def kernel(x, pos_bias, sinusoidal_pos, mask, W_qkv, b_qkv, W_o, b_o):
    global LAST_RESULT
    if TRACE:
        _ensure_profile_hook()
    in_maps, has_bias = _prep_inputs(x, pos_bias, sinusoidal_pos, mask,
                                     W_qkv, b_qkv, W_o, b_o)
    nc = _get_nc(has_bias)
    try:
        r = run_bass_kernel_spmd(nc, in_maps, list(range(NCORES)),
                                 trace=TRACE)
    except Exception:
        # occasional transient NRT device errors — retry once
        r = run_bass_kernel_spmd(nc, in_maps, list(range(NCORES)),
                                 trace=TRACE)
    LAST_RESULT = r
    acc = np.zeros((T, D), np.float64)
    for c in range(NCORES):
        acc += r.results[c]["out"].astype(np.float64)
    out = (acc + np.asarray(b_o, np.float32).astype(np.float64)).astype(
        np.float32)
    return out.reshape(B, S, D)

